# revision 4
# baseline (speedup 1.0000x reference)
"""Baichuan attention prefill (q_len=2048, H=5120, 40 heads) on 8 Trainium2
NeuronCores, tensor-parallel over heads (5 heads/core), all-reduce on host.

Per-core pipeline (all matmuls in fp32r = TF32):
  Phase 1: fused QKV projection.  qT/kT computed channel-major ([ch, tok]),
           v computed token-major ([tok, ch]).  Contraction dim (5120) is
           split in two SBUF-resident halves with DRAM accumulation.
  Phase 2: per-head attention.  scoresT[j,i] = kT_j^T @ qT_i  (PSUM), +maskT
           on DVE, exp on ACT (writing rounded fp32r), then
           attnT[d,i] += v_j^T @ e  and  sums[1,i] += ones^T @ e  on PE.
           Normalization: recip(sums) -> K=1-matmul partition broadcast ->
           DVE multiply into persistent attnT SBUF tiles.
  Phase 3: row-parallel o_proj from attnT; partial [2048, 5120] per core,
           summed across the 8 cores on the host.
"""

import math
import numpy as np

import concourse.bass as bass
import concourse.mybir as mybir
import concourse.tile as tile
from concourse import bacc
from concourse.bass_utils import run_bass_kernel_spmd

H = 5120
NH = 40
HD = 128
T = 2048
NCORES = 8
HPC = NH // NCORES          # 5 heads per core
DPC = HPC * HD              # 640 channels per core
KC = H // 128               # 40 contraction chunks
KHALF = KC // 2             # 20 per resident half

F32 = mybir.dt.float32
F32R = mybir.dt.float32r
EXP = mybir.ActivationFunctionType.Exp

MM_DT = F32R                # matmul operand dtype knob (F32R or F32)


def _phase1(nc, tc, xT, wqkvT, qk_d, v_d):
    """qkT ([1280, 2048], rows 0:640 = scaled qT, 640:1280 = kT) and
    v ([2048, 640]) via k-split halves with DRAM accumulation."""
    MT = 2 * DPC // 128     # 10 m-tiles for the qk block
    NT = T // 512           # 4 n-tiles
    IT = T // 128           # 16 token chunks for v
    for half in range(2):
        k0 = half * KHALF
        first = half == 0
        with tc.tile_pool(name=f"p1x{half}", bufs=1) as xp:
            xt = []
            for k in range(KHALF):
                xk = xp.tile([128, T], MM_DT, tag=f"x{k}", name=f"x_{half}_{k}")
                nc.sync.dma_start(out=xk, in_=xT[(k0 + k) * 128:(k0 + k + 1) * 128, :].bitcast(MM_DT))
                xt.append(xk)
            # ---- qkT block: lhsT = weight col tile, rhs = xT
            with tc.tile_pool(name=f"p1w{half}", bufs=1) as wp, \
                 tc.tile_pool(name=f"p1o{half}", bufs=3) as op, \
                 tc.tile_pool(name=f"p1r{half}", bufs=3) as rp, \
                 tc.tile_pool(name=f"p1ps{half}", bufs=2, space="PSUM") as pp:
                for m in range(MT):
                    wt = []
                    for k in range(KHALF):
                        wk = wp.tile([128, 128], MM_DT, tag=f"w{k}", name=f"w_{half}_{m}_{k}")
                        nc.sync.dma_start(
                            out=wk,
                            in_=wqkvT[(k0 + k) * 128:(k0 + k + 1) * 128, m * 128:(m + 1) * 128].bitcast(MM_DT))
                        wt.append(wk)
                    for n in range(NT):
                        ps = pp.tile([128, 512], F32, tag="qkps", name=f"qkps_{half}_{m}_{n}")
                        for k in range(KHALF):
                            nc.tensor.matmul(ps, wt[k], xt[k][:, n * 512:(n + 1) * 512],
                                             start=(k == 0), stop=(k == KHALF - 1))
                        ob = op.tile([128, 512], MM_DT, tag="qko", name=f"qko_{half}_{m}_{n}")
                        dst = qk_d[m * 128:(m + 1) * 128, n * 512:(n + 1) * 512]
                        if first:
                            nc.scalar.copy(ob, ps)
                        else:
                            rb = rp.tile([128, 512], F32, tag="qkr", name=f"qkr_{half}_{m}_{n}")
                            nc.sync.dma_start(out=rb, in_=dst.bitcast(F32))
                            nc.vector.tensor_add(ob, ps, rb)
                        nc.sync.dma_start(out=dst, in_=ob)
            # ---- v block (token-major): lhsT = xT col slice, rhs = wv col tile
            with tc.tile_pool(name=f"p1wv{half}", bufs=1) as wvp, \
                 tc.tile_pool(name=f"p1vo{half}", bufs=2) as vop, \
                 tc.tile_pool(name=f"p1vr{half}", bufs=2) as vrp, \
                 tc.tile_pool(name=f"p1vps{half}", bufs=2, space="PSUM") as vpp:
                for c in range(2):          # 640 = 2 x 320
                    wvt = []
                    for k in range(KHALF):
                        wvk = wvp.tile([128, 320], MM_DT, tag=f"wv{k}", name=f"wv_{half}_{c}_{k}")
                        nc.sync.dma_start(
                            out=wvk,
                            in_=wqkvT[(k0 + k) * 128:(k0 + k + 1) * 128,
                                      2 * DPC + c * 320:2 * DPC + (c + 1) * 320].bitcast(MM_DT))
                        wvt.append(wvk)
                    for i in range(IT):
                        ps = vpp.tile([128, 320], F32, tag="vps", name=f"vps_{half}_{c}_{i}")
                        for k in range(KHALF):
                            nc.tensor.matmul(ps, xt[k][:, i * 128:(i + 1) * 128], wvt[k],
                                             start=(k == 0), stop=(k == KHALF - 1))
                        ob = vop.tile([128, 320], MM_DT, tag="vo", name=f"vo_{half}_{c}_{i}")
                        dst = v_d[i * 128:(i + 1) * 128, c * 320:(c + 1) * 320]
                        if first:
                            nc.scalar.copy(ob, ps)
                        else:
                            rb = vrp.tile([128, 320], F32, tag="vr", name=f"vr_{half}_{c}_{i}")
                            nc.sync.dma_start(out=rb, in_=dst.bitcast(F32))
                            nc.vector.tensor_add(ob, ps, rb)
                        nc.sync.dma_start(out=dst, in_=ob)


def _phase2(nc, tc, qk_d, v_d, maskT, attnT, ones_col, ones_row):
    """Per-head fused attention into persistent attnT tiles."""
    ITN = T // 512          # 4 i-tiles
    JC = T // 128           # 16 j-chunks
    for h in range(HPC):
        with tc.tile_pool(name=f"p2h{h}", bufs=1) as hp, \
             tc.tile_pool(name=f"p2m{h}", bufs=8) as mp, \
             tc.tile_pool(name=f"p2e{h}", bufs=4) as ep, \
             tc.tile_pool(name=f"p2misc{h}", bufs=2) as msc, \
             tc.tile_pool(name=f"p2sc{h}", bufs=2, space="PSUM") as scp, \
             tc.tile_pool(name=f"p2acc{h}", bufs=2, space="PSUM") as accp:
            qT = hp.tile([128, T], MM_DT, name=f"qT_{h}")
            nc.sync.dma_start(out=qT, in_=qk_d[h * 128:(h + 1) * 128, :])
            kT = hp.tile([128, T], MM_DT, name=f"kT_{h}")
            nc.sync.dma_start(out=kT, in_=qk_d[DPC + h * 128:DPC + (h + 1) * 128, :])
            v_tiles = []
            for j in range(JC):
                vj = hp.tile([128, 128], MM_DT, name=f"v_{h}_{j}")
                nc.sync.dma_start(out=vj, in_=v_d[j * 128:(j + 1) * 128, h * 128:(h + 1) * 128])
                v_tiles.append(vj)
            for it in range(ITN):
                attn_ps = accp.tile([128, 512], F32, tag="attnps", name=f"attnps_{h}_{it}")
                sum_ps = accp.tile([1, 512], F32, tag="sumps", name=f"sumps_{h}_{it}")
                for j in range(JC):
                    sc = scp.tile([128, 512], F32, tag="scps", name=f"scps_{h}_{it}_{j}")
                    nc.tensor.matmul(sc, kT[:, j * 128:(j + 1) * 128],
                                     qT[:, it * 512:(it + 1) * 512], start=True, stop=True)
                    mt = mp.tile([128, 512], F32, tag="mask", name=f"mask_{h}_{it}_{j}")
                    nc.sync.dma_start(
                        out=mt, in_=maskT[h, j * 128:(j + 1) * 128, it * 512:(it + 1) * 512])
                    ts = ep.tile([128, 512], F32, tag="t", name=f"t_{h}_{it}_{j}")
                    nc.vector.tensor_add(ts, sc, mt)
                    et = ep.tile([128, 512], MM_DT, tag="e", name=f"e_{h}_{it}_{j}")
                    nc.scalar.activation(et, ts, EXP)
                    nc.tensor.matmul(attn_ps, v_tiles[j], et,
                                     start=(j == 0), stop=(j == JC - 1))
                    nc.tensor.matmul(sum_ps, ones_col, et,
                                     start=(j == 0), stop=(j == JC - 1))
                rec = msc.tile([1, 512], F32, tag="rec", name=f"rec_{h}_{it}")
                nc.vector.reciprocal(rec, sum_ps)
                rec_r = msc.tile([1, 512], MM_DT, tag="recr", name=f"recr_{h}_{it}")
                nc.vector.tensor_copy(rec_r, rec)
                bc_ps = scp.tile([128, 512], F32, tag="bcps", name=f"bcps_{h}_{it}", bufs=1)
                nc.tensor.matmul(bc_ps, ones_row, rec_r, start=True, stop=True)
                bc = msc.tile([128, 512], F32, tag="bc", name=f"bc_{h}_{it}")
                nc.scalar.copy(bc, bc_ps)
                nc.vector.tensor_mul(attnT[h][:, it * 512:(it + 1) * 512], attn_ps, bc)


def _phase3(nc, tc, attnT, woT, y):
    """Row-parallel o_proj: y_partial[i, o] = sum_dh attnT[dh, i] * woT[dh, o]."""
    OT = H // 512           # 10 output column tiles
    IT = T // 128           # 16 token chunks
    with tc.tile_pool(name="p3w", bufs=2) as wp, \
         tc.tile_pool(name="p3o", bufs=4) as op, \
         tc.tile_pool(name="p3ps", bufs=3, space="PSUM") as pp:
        for o in range(OT):
            wt = []
            for c in range(HPC):
                wc = wp.tile([128, 512], MM_DT, tag=f"wo{c}", name=f"wo_{o}_{c}")
                nc.sync.dma_start(
                    out=wc, in_=woT[c * 128:(c + 1) * 128, o * 512:(o + 1) * 512].bitcast(MM_DT))
                wt.append(wc)
            for i in range(IT):
                ps = pp.tile([128, 512], F32, tag="yps", name=f"yps_{o}_{i}")
                for c in range(HPC):
                    nc.tensor.matmul(ps, attnT[c][:, i * 128:(i + 1) * 128], wt[c],
                                     start=(c == 0), stop=(c == HPC - 1))
                ob = op.tile([128, 512], F32, tag="yo", name=f"yo_{o}_{i}")
                nc.scalar.copy(ob, ps)
                nc.sync.dma_start(out=y[i * 128:(i + 1) * 128, o * 512:(o + 1) * 512], in_=ob)


def build():
    nc = bacc.Bacc("TRN2", target_bir_lowering=False, debug=False, num_devices=NCORES)
    xT = nc.dram_tensor("xT", [H, T], F32, kind="ExternalInput").ap()
    wqkvT = nc.dram_tensor("wqkvT", [H, 3 * DPC], F32, kind="ExternalInput").ap()
    woT = nc.dram_tensor("woT", [DPC, H], F32, kind="ExternalInput").ap()
    maskT = nc.dram_tensor("maskT", [HPC, T, T], F32, kind="ExternalInput").ap()
    y = nc.dram_tensor("y", [T, H], F32, kind="ExternalOutput").ap()

    with tile.TileContext(nc) as tc:
        with tc.tile_pool(name="dramp", bufs=1, space="DRAM") as dp:
            qk_d = dp.tile([2 * DPC, T], MM_DT, name="qk_d")
            v_d = dp.tile([T, DPC], MM_DT, name="v_d")
            _phase1(nc, tc, xT, wqkvT, qk_d, v_d)
            with tc.tile_pool(name="attnTp", bufs=1) as ap, \
                 tc.tile_pool(name="constp", bufs=1) as cp:
                ones_f = cp.tile([128, 128], F32, name="ones_f")
                nc.vector.memset(ones_f, 1.0)
                ones_col = cp.tile([128, 1], MM_DT, name="ones_col")
                nc.vector.tensor_copy(ones_col, ones_f[:, 0:1])
                ones_row = cp.tile([1, 128], MM_DT, name="ones_row")
                nc.vector.tensor_copy(ones_row, ones_f[0:1, :])
                attnT = [ap.tile([128, T], MM_DT, name=f"attnT_{c}") for c in range(HPC)]
                _phase2(nc, tc, qk_d, v_d, maskT, attnT, ones_col, ones_row)
                _phase3(nc, tc, attnT, woT, y)
    nc.compile()
    return nc


_nc = None


def _get_nc():
    global _nc
    if _nc is None:
        _nc = build()
    return _nc


def make_in_maps(hidden_states, attention_mask, W_pack, o_proj_w):
    hs = np.ascontiguousarray(np.asarray(hidden_states, dtype=np.float32).reshape(T, H))
    mask = np.asarray(attention_mask, dtype=np.float32)
    wp = np.asarray(W_pack, dtype=np.float32)
    wo = np.asarray(o_proj_w, dtype=np.float32)

    xT = np.ascontiguousarray(hs.T)                       # [H, T]
    scale = np.float32(1.0 / math.sqrt(HD))
    wq = wp[0:H].reshape(NH, HD, H)
    wk = wp[H:2 * H].reshape(NH, HD, H)
    wv = wp[2 * H:3 * H].reshape(NH, HD, H)

    in_maps = []
    for c in range(NCORES):
        h0, h1 = c * HPC, (c + 1) * HPC
        w_c = np.concatenate([
            wq[h0:h1].reshape(DPC, H) * scale,
            wk[h0:h1].reshape(DPC, H),
            wv[h0:h1].reshape(DPC, H),
        ], axis=0)                                        # [1920, H]
        wqkvT_c = np.ascontiguousarray(w_c.T)             # [H, 1920]
        woT_c = np.ascontiguousarray(wo[:, h0 * HD:h1 * HD].T)    # [640, H]
        maskT_c = np.ascontiguousarray(mask[h0:h1].transpose(0, 2, 1))  # [5, T, T]
        in_maps.append({"xT": xT, "wqkvT": wqkvT_c, "woT": woT_c, "maskT": maskT_c})
    return in_maps


def kernel(input_pos=None, end=None, hidden_states=None, attention_mask=None,
           W_pack=None, o_proj_w=None, k_cache=None, v_cache=None):
    # input_pos == arange(T) and end == T per the problem spec, so the KV
    # cache write is a full overwrite and the zero-filled caches never
    # contribute to the output — both are intentionally unused here.
    in_maps = make_in_maps(hidden_states, attention_mask, W_pack, o_proj_w)
    nc = _get_nc()
    res = run_bass_kernel_spmd(nc, in_maps, list(range(NCORES)))
    y = res.results[0]["y"].astype(np.float32)
    for c in range(1, NCORES):
        y = y + res.results[c]["y"]
    return y.reshape(1, T, H)


# revision 8
# speedup vs baseline: 7.3369x; 7.3369x over previous
"""Baichuan attention prefill (q_len=2048, H=5120, 40 heads) on 8 Trainium2
NeuronCores, tensor-parallel over heads (5 heads/core), all-reduce on host.

Per-core pipeline (all matmuls in fp32r = TF32):
  Phase 1: fused QKV projection.  qT/kT computed channel-major ([ch, tok]),
           v computed token-major ([tok, ch]).  Contraction dim (5120) is
           split in two SBUF-resident halves with DRAM accumulation.
  Phase 2: per-head attention.  scoresT[j,i] = kT_j^T @ qT_i  (PSUM), +maskT
           on DVE, exp on ACT (writing rounded fp32r), then
           attnT[d,i] += v_j^T @ e  and  sums[1,i] += ones^T @ e  on PE.
           Normalization: recip(sums) -> K=1-matmul partition broadcast ->
           DVE multiply into persistent attnT SBUF tiles.
  Phase 3: row-parallel o_proj from attnT; partial [2048, 5120] per core,
           summed across the 8 cores on the host.

DMA queue assignment (separate HW-DGE queues so streams don't serialize):
  sync   - x tiles, mask tiles
  scalar - all weight loads + per-head qT/kT/v loads
  vector - phase-1 partial readbacks
  gpsimd - all stores to DRAM
"""

import math
import numpy as np

import concourse.bass as bass
import concourse.mybir as mybir
import concourse.tile as tile
from concourse import bacc
from concourse.bass_utils import run_bass_kernel_spmd

H = 5120
NH = 40
HD = 128
T = 2048
NCORES = 8
HPC = NH // NCORES          # 5 heads per core
DPC = HPC * HD              # 640 channels per core
KC = H // 128               # 40 contraction chunks
KHALF = KC // 2             # 20 per resident half

F32 = mybir.dt.float32
F16 = mybir.dt.float16
F32R = mybir.dt.float32r
EXP = mybir.ActivationFunctionType.Exp

MM_DT = F32R                # matmul operand dtype knob (F32R or F32)
MASK_DT = F16               # attention-mask stream dtype (F16 halves DMA)


def _phase1(nc, tc, xT, wqkvT, qk_d, v_d):
    """qkT ([1280, 2048], rows 0:640 = scaled qT, 640:1280 = kT) and
    v ([2048, 640]) via k-split halves with DRAM accumulation."""
    MT = 2 * DPC // 128     # 10 m-tiles for the qk block
    NT = T // 512           # 4 n-tiles
    IT = T // 128           # 16 token chunks for v
    for half in range(2):
        k0 = half * KHALF
        first = half == 0
        with tc.tile_pool(name=f"p1x{half}", bufs=1) as xp:
            xt = []
            for k in range(KHALF):
                xk = xp.tile([128, T], MM_DT, tag=f"x{k}", name=f"x_{half}_{k}")
                nc.sync.dma_start(out=xk, in_=xT[(k0 + k) * 128:(k0 + k + 1) * 128, :].bitcast(MM_DT))
                xt.append(xk)
            # ---- qkT block: lhsT = weight col tile, rhs = xT
            with tc.tile_pool(name=f"p1w{half}", bufs=2) as wp, \
                 tc.tile_pool(name=f"p1o{half}", bufs=3) as op, \
                 tc.tile_pool(name=f"p1r{half}", bufs=3) as rp, \
                 tc.tile_pool(name=f"p1ps{half}", bufs=2, space="PSUM") as pp:
                for m in range(MT):
                    wt = []
                    for k in range(KHALF):
                        wk = wp.tile([128, 128], MM_DT, tag=f"w{k}", name=f"w_{half}_{m}_{k}")
                        nc.scalar.dma_start(
                            out=wk,
                            in_=wqkvT[(k0 + k) * 128:(k0 + k + 1) * 128, m * 128:(m + 1) * 128].bitcast(MM_DT))
                        wt.append(wk)
                    for n in range(NT):
                        ps = pp.tile([128, 512], F32, tag="qkps", name=f"qkps_{half}_{m}_{n}")
                        for k in range(KHALF):
                            nc.tensor.matmul(ps, wt[k], xt[k][:, n * 512:(n + 1) * 512],
                                             start=(k == 0), stop=(k == KHALF - 1))
                        ob = op.tile([128, 512], MM_DT, tag="qko", name=f"qko_{half}_{m}_{n}")
                        dst = qk_d[m * 128:(m + 1) * 128, n * 512:(n + 1) * 512]
                        if first:
                            nc.scalar.copy(ob, ps)
                        else:
                            rb = rp.tile([128, 512], F32, tag="qkr", name=f"qkr_{half}_{m}_{n}")
                            nc.gpsimd.dma_start(out=rb, in_=dst.bitcast(F32))
                            nc.vector.tensor_add(ob, ps, rb)
                        nc.gpsimd.dma_start(out=dst, in_=ob)
            # ---- v block (token-major): lhsT = xT col slice, rhs = wv col tile
            with tc.tile_pool(name=f"p1wv{half}", bufs=1) as wvp, \
                 tc.tile_pool(name=f"p1vo{half}", bufs=2) as vop, \
                 tc.tile_pool(name=f"p1vr{half}", bufs=2) as vrp, \
                 tc.tile_pool(name=f"p1vps{half}", bufs=2, space="PSUM") as vpp:
                for c in range(2):          # 640 = 2 x 320
                    wvt = []
                    for k in range(KHALF):
                        wvk = wvp.tile([128, 320], MM_DT, tag=f"wv{k}", name=f"wv_{half}_{c}_{k}")
                        nc.scalar.dma_start(
                            out=wvk,
                            in_=wqkvT[(k0 + k) * 128:(k0 + k + 1) * 128,
                                      2 * DPC + c * 320:2 * DPC + (c + 1) * 320].bitcast(MM_DT))
                        wvt.append(wvk)
                    for i in range(IT):
                        ps = vpp.tile([128, 320], F32, tag="vps", name=f"vps_{half}_{c}_{i}")
                        for k in range(KHALF):
                            nc.tensor.matmul(ps, xt[k][:, i * 128:(i + 1) * 128], wvt[k],
                                             start=(k == 0), stop=(k == KHALF - 1))
                        ob = vop.tile([128, 320], MM_DT, tag="vo", name=f"vo_{half}_{c}_{i}")
                        dst = v_d[i * 128:(i + 1) * 128, c * 320:(c + 1) * 320]
                        if first:
                            nc.scalar.copy(ob, ps)
                        else:
                            rb = vrp.tile([128, 320], F32, tag="vr", name=f"vr_{half}_{c}_{i}")
                            nc.gpsimd.dma_start(out=rb, in_=dst.bitcast(F32))
                            nc.vector.tensor_add(ob, ps, rb)
                        nc.gpsimd.dma_start(out=dst, in_=ob)


def _phase2(nc, tc, qk_d, v_d, maskT, attnT, ones_col):
    """Per-head fused attention into persistent attnT tiles.  All pools span
    the head loop; per-head q/k/v tiles alternate tags (h%2) so head h+1's
    loads prefetch during head h's compute."""
    ITN = T // 512          # 4 i-tiles
    JC = T // 128           # 16 j-chunks
    with tc.tile_pool(name="p2h", bufs=1) as hp, \
         tc.tile_pool(name="p2m", bufs=12) as mp, \
         tc.tile_pool(name="p2t", bufs=4) as tp_, \
         tc.tile_pool(name="p2e", bufs=4) as ep, \
         tc.tile_pool(name="p2misc", bufs=2) as msc, \
         tc.tile_pool(name="p2sc", bufs=3, space="PSUM") as scp, \
         tc.tile_pool(name="p2acc", bufs=2, space="PSUM") as accp:
        for h in range(HPC):
            par = h % 2
            qT = hp.tile([128, T], MM_DT, tag=f"qT{par}", name=f"qT_{h}")
            nc.scalar.dma_start(out=qT, in_=qk_d[h * 128:(h + 1) * 128, :])
            kT = hp.tile([128, T], MM_DT, tag=f"kT{par}", name=f"kT_{h}")
            nc.scalar.dma_start(out=kT, in_=qk_d[DPC + h * 128:DPC + (h + 1) * 128, :])
            v_tiles = []
            for j in range(JC):
                vj = hp.tile([128, 128], MM_DT, tag=f"v{j}_{par}", name=f"v_{h}_{j}")
                nc.scalar.dma_start(out=vj, in_=v_d[j * 128:(j + 1) * 128, h * 128:(h + 1) * 128])
                v_tiles.append(vj)
            for it in range(ITN):
                attn_ps = accp.tile([128, 512], F32, tag="attnps", name=f"attnps_{h}_{it}")
                sum_ps = accp.tile([1, 512], F32, tag="sumps", name=f"sumps_{h}_{it}")
                for j in range(JC):
                    sc = scp.tile([128, 512], F32, tag="scps", name=f"scps_{h}_{it}_{j}")
                    nc.tensor.matmul(sc, kT[:, j * 128:(j + 1) * 128],
                                     qT[:, it * 512:(it + 1) * 512], start=True, stop=True)
                    mt = mp.tile([128, 512], MASK_DT, tag="mask", name=f"mask_{h}_{it}_{j}")
                    nc.sync.dma_start(
                        out=mt, in_=maskT[h, j * 128:(j + 1) * 128, it * 512:(it + 1) * 512])
                    ts = tp_.tile([128, 512], F32, tag="t", name=f"t_{h}_{it}_{j}")
                    nc.vector.tensor_add(ts, sc, mt)
                    et = ep.tile([128, 512], MM_DT, tag="e", name=f"e_{h}_{it}_{j}")
                    nc.scalar.activation(et, ts, EXP)
                    nc.tensor.matmul(attn_ps, v_tiles[j], et,
                                     start=(j == 0), stop=(j == JC - 1))
                    nc.tensor.matmul(sum_ps, ones_col, et,
                                     start=(j == 0), stop=(j == JC - 1))
                rec = msc.tile([1, 512], F32, tag="rec", name=f"rec_{h}_{it}")
                nc.vector.reciprocal(rec, sum_ps)
                bc = msc.tile([128, 512], F32, tag="bc", name=f"bc_{h}_{it}")
                nc.gpsimd.partition_broadcast(bc, rec)
                nc.vector.tensor_mul(attnT[h][:, it * 512:(it + 1) * 512], attn_ps, bc)


def _phase3(nc, tc, attnT, woT, y):
    """Row-parallel o_proj: y_partial[i, o] = sum_dh attnT[dh, i] * woT[dh, o]."""
    OT = H // 512           # 10 output column tiles
    IT = T // 128           # 16 token chunks
    with tc.tile_pool(name="p3w", bufs=2) as wp, \
         tc.tile_pool(name="p3o", bufs=4) as op, \
         tc.tile_pool(name="p3ps", bufs=3, space="PSUM") as pp:
        for o in range(OT):
            wt = []
            for c in range(HPC):
                wc = wp.tile([128, 512], MM_DT, tag=f"wo{c}", name=f"wo_{o}_{c}")
                nc.scalar.dma_start(
                    out=wc, in_=woT[c * 128:(c + 1) * 128, o * 512:(o + 1) * 512].bitcast(MM_DT))
                wt.append(wc)
            for i in range(IT):
                ps = pp.tile([128, 512], F32, tag="yps", name=f"yps_{o}_{i}")
                for c in range(HPC):
                    nc.tensor.matmul(ps, attnT[c][:, i * 128:(i + 1) * 128], wt[c],
                                     start=(c == 0), stop=(c == HPC - 1))
                ob = op.tile([128, 512], F32, tag="yo", name=f"yo_{o}_{i}")
                nc.scalar.copy(ob, ps)
                nc.gpsimd.dma_start(out=y[i * 128:(i + 1) * 128, o * 512:(o + 1) * 512], in_=ob)


def build():
    nc = bacc.Bacc("TRN2", target_bir_lowering=False, debug=False, num_devices=NCORES)
    xT = nc.dram_tensor("xT", [H, T], F32, kind="ExternalInput").ap()
    wqkvT = nc.dram_tensor("wqkvT", [H, 3 * DPC], F32, kind="ExternalInput").ap()
    woT = nc.dram_tensor("woT", [DPC, H], F32, kind="ExternalInput").ap()
    maskT = nc.dram_tensor("maskT", [HPC, T, T], MASK_DT, kind="ExternalInput").ap()
    y = nc.dram_tensor("y", [T, H], F32, kind="ExternalOutput").ap()

    with tile.TileContext(nc) as tc:
        with tc.tile_pool(name="dramp", bufs=1, space="DRAM") as dp:
            qk_d = dp.tile([2 * DPC, T], MM_DT, name="qk_d")
            v_d = dp.tile([T, DPC], MM_DT, name="v_d")
            _phase1(nc, tc, xT, wqkvT, qk_d, v_d)
            with tc.tile_pool(name="attnTp", bufs=1) as ap, \
                 tc.tile_pool(name="constp", bufs=1) as cp:
                ones_f = cp.tile([128, 128], F32, name="ones_f")
                nc.vector.memset(ones_f, 1.0)
                ones_col = cp.tile([128, 1], MM_DT, name="ones_col")
                nc.vector.tensor_copy(ones_col, ones_f[:, 0:1])
                attnT = [ap.tile([128, T], MM_DT, name=f"attnT_{c}") for c in range(HPC)]
                _phase2(nc, tc, qk_d, v_d, maskT, attnT, ones_col)
                _phase3(nc, tc, attnT, woT, y)
    nc.compile()
    return nc


_nc = None


def _get_nc():
    global _nc
    if _nc is None:
        _nc = build()
    return _nc


def make_in_maps(hidden_states, attention_mask, W_pack, o_proj_w):
    hs = np.ascontiguousarray(np.asarray(hidden_states, dtype=np.float32).reshape(T, H))
    mask = np.asarray(attention_mask, dtype=np.float32)
    wp = np.asarray(W_pack, dtype=np.float32)
    wo = np.asarray(o_proj_w, dtype=np.float32)

    xT = np.ascontiguousarray(hs.T)                       # [H, T]
    scale = np.float32(1.0 / math.sqrt(HD))
    wq = wp[0:H].reshape(NH, HD, H)
    wk = wp[H:2 * H].reshape(NH, HD, H)
    wv = wp[2 * H:3 * H].reshape(NH, HD, H)
    mask_np_dt = np.float16 if MASK_DT == F16 else np.float32

    in_maps = []
    for c in range(NCORES):
        h0, h1 = c * HPC, (c + 1) * HPC
        w_c = np.concatenate([
            wq[h0:h1].reshape(DPC, H) * scale,
            wk[h0:h1].reshape(DPC, H),
            wv[h0:h1].reshape(DPC, H),
        ], axis=0)                                        # [1920, H]
        wqkvT_c = np.ascontiguousarray(w_c.T)             # [H, 1920]
        woT_c = np.ascontiguousarray(wo[:, h0 * HD:h1 * HD].T)    # [640, H]
        maskT_c = np.ascontiguousarray(
            mask[h0:h1].transpose(0, 2, 1).astype(mask_np_dt))    # [5, T, T]
        in_maps.append({"xT": xT, "wqkvT": wqkvT_c, "woT": woT_c, "maskT": maskT_c})
    return in_maps


def kernel(input_pos=None, end=None, hidden_states=None, attention_mask=None,
           W_pack=None, o_proj_w=None, k_cache=None, v_cache=None):
    # input_pos == arange(T) and end == T per the problem spec, so the KV
    # cache write is a full overwrite and the zero-filled caches never
    # contribute to the output — both are intentionally unused here.
    in_maps = make_in_maps(hidden_states, attention_mask, W_pack, o_proj_w)
    nc = _get_nc()
    res = run_bass_kernel_spmd(nc, in_maps, list(range(NCORES)))
    y = res.results[0]["y"].astype(np.float32)
    for c in range(1, NCORES):
        y = y + res.results[c]["y"]
    return y.reshape(1, T, H)


# revision 14
# speedup vs baseline: 9.4545x; 1.2886x over previous
"""Baichuan attention prefill (q_len=2048, H=5120, 40 heads) on 8 Trainium2
NeuronCores, tensor-parallel over heads (5 heads/core), all-reduce on host.

Per-core pipeline (matmuls in fp32r = TF32 except the f16 o_proj):
  Phase 1: fused QKV projection, channel-major: qkvT [1920, 2048] = W_c @ X^T.
           Contraction dim (5120) is split in two SBUF-resident halves; the
           second half accumulates into DRAM via SWDGE accumulate-DMA.
           m-tiles are emitted head-interleaved (q_h, k_h, v_h, q_h+1, ...)
           so phase 2 can start on head h as soon as its three rows land.
  Phase 2: per-head attention.  v_h is recovered token-major by 16 PE
           transposes of vT_h.  scoresT[j,i] = kT_j^T @ qT_i (PSUM), +maskT
           (f16 stream) on DVE, exp on ACT (writes rounded fp32r), then
           attnT[d,i] += v_j^T @ e  and  sums[1,i] += ones^T @ e  on PE.
           Normalization: recip(sums) -> gpsimd partition_broadcast ->
           DVE multiply into persistent f16 attnT SBUF tiles.
  Phase 3: row-parallel o_proj from f16 attnT with f16 weights; partial
           [2048, 5120] f32 per core, summed across the 8 cores on host.

DMA notes: descriptors cost ~0.5-1us issue each, so transfers are batched
into few large 3D-AP descriptors.  Queue split: sync = x+mask streams,
scalar = weights + per-head loads, gpsimd(SWDGE) = stores/accumulates.
"""

import math
import numpy as np

import concourse.bass as bass
import concourse.mybir as mybir
import concourse.tile as tile
from concourse import bacc
from concourse.bass_utils import run_bass_kernel_spmd
from concourse.masks import make_identity

H = 5120
NH = 40
HD = 128
T = 2048
NCORES = 8
HPC = NH // NCORES          # 5 heads per core
DPC = HPC * HD              # 640 channels per core
KC = H // 128               # 40 contraction chunks
KHALF = KC // 2             # 20 per resident half

F32 = mybir.dt.float32
F16 = mybir.dt.float16
F32R = mybir.dt.float32r
EXP = mybir.ActivationFunctionType.Exp
ADD = mybir.AluOpType.add

MM_DT = F32R                # qkv/attention matmul operand dtype
O_DT = F16                  # o_proj operand dtype (attnT + wo)
MASK_DT = F16               # attention-mask stream dtype

# head-interleaved m-tile order: q_h, k_h, v_h for h = 0..HPC-1
M_ORDER = [b * HPC + h for h in range(HPC) for b in range(3)]


def _phase1(nc, tc, xT, wqkvT, qkv_d):
    """qkvT [1920, 2048] = per-core [scaled q; k; v] channel-major, via
    k-split halves; half 1 accumulates into DRAM with SWDGE accum-DMA."""
    NT = T // 512           # 4 n-tiles
    for half in range(2):
        k0 = half * KHALF
        first = half == 0
        with tc.tile_pool(name=f"p1x{half}", bufs=1) as xp, \
             tc.tile_pool(name=f"p1w{half}", bufs=2) as wp, \
             tc.tile_pool(name=f"p1o{half}", bufs=2) as op, \
             tc.tile_pool(name=f"p1ps{half}", bufs=2, space="PSUM") as pp:
            xt = []
            for k in range(KHALF):
                xk = xp.tile([128, T], MM_DT, tag=f"x{k}", name=f"x_{half}_{k}")
                nc.sync.dma_start(out=xk, in_=xT[(k0 + k) * 128:(k0 + k + 1) * 128, :].bitcast(MM_DT))
                xt.append(xk)
            for m in M_ORDER:
                wm = wp.tile([128, KHALF, 128], MM_DT, tag="w", name=f"w_{half}_{m}")
                nc.scalar.dma_start(
                    out=wm,
                    in_=wqkvT[k0 * 128:(k0 + KHALF) * 128, m * 128:(m + 1) * 128]
                    .rearrange("(k p) m -> p k m", p=128).bitcast(MM_DT))
                ob = op.tile([128, T], F32, tag="qko", name=f"qko_{half}_{m}")
                for n in range(NT):
                    ps = pp.tile([128, 512], F32, tag="qkps", name=f"qkps_{half}_{m}_{n}")
                    for k in range(KHALF):
                        nc.tensor.matmul(ps, wm[:, k, :], xt[k][:, n * 512:(n + 1) * 512],
                                         start=(k == 0), stop=(k == KHALF - 1))
                    nc.scalar.copy(ob[:, n * 512:(n + 1) * 512], ps)
                dst = qkv_d[m * 128:(m + 1) * 128, :]
                if first:
                    nc.gpsimd.dma_start(out=dst, in_=ob)
                else:
                    nc.gpsimd.dma_start(out=dst, in_=ob, accum_op=ADD)


def _phase2(nc, tc, qkv_d, maskT, attnT, ones_col, ident_f):
    """Per-head fused attention into persistent attnT tiles.  All pools span
    the head loop; per-head q/k/v tiles alternate tags (h%2) so head h+1's
    loads prefetch during head h's compute.  Mask streams one [128, 2048]
    f16 tile per (head, j-chunk)."""
    ITN = T // 512          # 4 i-tiles
    JC = T // 128           # 16 j-chunks
    with tc.tile_pool(name="p2h", bufs=1) as hp, \
         tc.tile_pool(name="p2m", bufs=1) as mp, \
         tc.tile_pool(name="p2t", bufs=4) as tp_, \
         tc.tile_pool(name="p2e", bufs=4) as ep, \
         tc.tile_pool(name="p2misc", bufs=2) as msc, \
         tc.tile_pool(name="p2sc", bufs=3, space="PSUM") as scp, \
         tc.tile_pool(name="p2acc", bufs=2, space="PSUM") as accp:
        for h in range(HPC):
            par = h % 2
            qT = hp.tile([128, T], MM_DT, tag=f"qT{par}", name=f"qT_{h}")
            nc.scalar.dma_start(out=qT, in_=qkv_d[h * 128:(h + 1) * 128, :].bitcast(MM_DT))
            kT = hp.tile([128, T], MM_DT, tag=f"kT{par}", name=f"kT_{h}")
            nc.scalar.dma_start(out=kT, in_=qkv_d[DPC + h * 128:DPC + (h + 1) * 128, :].bitcast(MM_DT))
            vT = hp.tile([128, T], F32, tag=f"vT{par}", name=f"vT_{h}")
            nc.scalar.dma_start(out=vT, in_=qkv_d[2 * DPC + h * 128:2 * DPC + (h + 1) * 128, :])
            v_tiles = []
            for j in range(JC):
                vt_ps = accp.tile([128, 128], F32, tag="vtps", name=f"vtps_{h}_{j}", bufs=1)
                nc.tensor.transpose(vt_ps, vT[:, j * 128:(j + 1) * 128], ident_f)
                vj = hp.tile([128, 128], MM_DT, tag=f"v{j}_{par}", name=f"v_{h}_{j}")
                nc.scalar.copy(vj, vt_ps)
                v_tiles.append(vj)
            mts = []
            for jc in range(JC):
                mt = mp.tile([128, T], MASK_DT, tag=f"m{jc}", name=f"mask_{h}_{jc}")
                nc.sync.dma_start(out=mt, in_=maskT[h, jc * 128:(jc + 1) * 128, :])
                mts.append(mt)
            for it in range(ITN):
                attn_ps = accp.tile([128, 512], F32, tag="attnps", name=f"attnps_{h}_{it}")
                sum_ps = accp.tile([1, 512], F32, tag="sumps", name=f"sumps_{h}_{it}")
                for j in range(JC):
                    sc = scp.tile([128, 512], F32, tag="scps", name=f"scps_{h}_{it}_{j}")
                    nc.tensor.matmul(sc, kT[:, j * 128:(j + 1) * 128],
                                     qT[:, it * 512:(it + 1) * 512], start=True, stop=True)
                    ts = tp_.tile([128, 512], F32, tag="t", name=f"t_{h}_{it}_{j}")
                    nc.vector.tensor_add(ts, sc, mts[j][:, it * 512:(it + 1) * 512])
                    et = ep.tile([128, 512], MM_DT, tag="e", name=f"e_{h}_{it}_{j}")
                    nc.scalar.activation(et, ts, EXP)
                    nc.tensor.matmul(attn_ps, v_tiles[j], et,
                                     start=(j == 0), stop=(j == JC - 1))
                    nc.tensor.matmul(sum_ps, ones_col, et,
                                     start=(j == 0), stop=(j == JC - 1))
                rec = msc.tile([1, 512], F32, tag="rec", name=f"rec_{h}_{it}")
                nc.vector.reciprocal(rec, sum_ps)
                bc = msc.tile([128, 512], F32, tag="bc", name=f"bc_{h}_{it}")
                nc.gpsimd.partition_broadcast(bc, rec)
                nc.vector.tensor_mul(attnT[h][:, it * 512:(it + 1) * 512], attn_ps, bc)


def _phase3(nc, tc, attnT, woT, y):
    """Row-parallel o_proj in f16: y_partial[i, o] = sum_dh attnT[dh, i]*wo[dh, o]."""
    OT = H // 512           # 10 output column tiles
    y3 = y.rearrange("(ic p) o -> p ic o", p=128)           # [128, 16, 5120]
    with tc.tile_pool(name="p3w", bufs=2) as wp, \
         tc.tile_pool(name="p3o", bufs=2) as op, \
         tc.tile_pool(name="p3ps", bufs=3, space="PSUM") as pp:
        for o in range(OT):
            wo = wp.tile([128, HPC, 512], O_DT, tag="wo", name=f"wo_{o}")
            nc.scalar.dma_start(
                out=wo,
                in_=woT[:, o * 512:(o + 1) * 512].rearrange("(c p) o -> p c o", p=128))
            for i4 in range(4):
                yo = op.tile([128, 4, 512], F32, tag="yo", name=f"yo_{o}_{i4}")
                for u in range(4):
                    i = i4 * 4 + u
                    ps = pp.tile([128, 512], F32, tag="yps", name=f"yps_{o}_{i}")
                    for c in range(HPC):
                        nc.tensor.matmul(ps, attnT[c][:, i * 128:(i + 1) * 128], wo[:, c, :],
                                         start=(c == 0), stop=(c == HPC - 1))
                    nc.scalar.copy(yo[:, u, :], ps)
                nc.gpsimd.dma_start(
                    out=y3[:, i4 * 4:(i4 + 1) * 4, o * 512:(o + 1) * 512], in_=yo)


def build():
    nc = bacc.Bacc("TRN2", target_bir_lowering=False, debug=False, num_devices=NCORES)
    xT = nc.dram_tensor("xT", [H, T], F32, kind="ExternalInput").ap()
    wqkvT = nc.dram_tensor("wqkvT", [H, 3 * DPC], F32, kind="ExternalInput").ap()
    woT = nc.dram_tensor("woT", [DPC, H], O_DT, kind="ExternalInput").ap()
    maskT = nc.dram_tensor("maskT", [HPC, T, T], MASK_DT, kind="ExternalInput").ap()
    y = nc.dram_tensor("y", [T, H], F32, kind="ExternalOutput").ap()

    with tile.TileContext(nc) as tc:
        with tc.tile_pool(name="dramp", bufs=1, space="DRAM") as dp:
            qkv_d = dp.tile([3 * DPC, T], F32, name="qkv_d")
            _phase1(nc, tc, xT, wqkvT, qkv_d)
            with tc.tile_pool(name="attnTp", bufs=1) as ap, \
                 tc.tile_pool(name="constp", bufs=1) as cp:
                ones_f = cp.tile([128, 1], F32, name="ones_f")
                nc.vector.memset(ones_f, 1.0)
                ones_col = cp.tile([128, 1], MM_DT, name="ones_col")
                nc.vector.tensor_copy(ones_col, ones_f)
                ident_f = cp.tile([128, 128], F32, name="ident_f")
                make_identity(nc, ident_f)
                attnT = [ap.tile([128, T], O_DT, name=f"attnT_{c}") for c in range(HPC)]
                _phase2(nc, tc, qkv_d, maskT, attnT, ones_col, ident_f)
                _phase3(nc, tc, attnT, woT, y)
    nc.compile()
    return nc


_nc = None


def _get_nc():
    global _nc
    if _nc is None:
        _nc = build()
    return _nc


def make_in_maps(hidden_states, attention_mask, W_pack, o_proj_w):
    hs = np.ascontiguousarray(np.asarray(hidden_states, dtype=np.float32).reshape(T, H))
    mask = np.asarray(attention_mask, dtype=np.float32)
    wp = np.asarray(W_pack, dtype=np.float32)
    wo = np.asarray(o_proj_w, dtype=np.float32)

    xT = np.ascontiguousarray(hs.T)                       # [H, T]
    scale = np.float32(1.0 / math.sqrt(HD))
    wq = wp[0:H].reshape(NH, HD, H)
    wk = wp[H:2 * H].reshape(NH, HD, H)
    wv = wp[2 * H:3 * H].reshape(NH, HD, H)
    mask_np_dt = np.float16 if MASK_DT == F16 else np.float32
    wo_np_dt = np.float16 if O_DT == F16 else np.float32

    in_maps = []
    for c in range(NCORES):
        h0, h1 = c * HPC, (c + 1) * HPC
        w_c = np.concatenate([
            wq[h0:h1].reshape(DPC, H) * scale,
            wk[h0:h1].reshape(DPC, H),
            wv[h0:h1].reshape(DPC, H),
        ], axis=0)                                        # [1920, H]
        wqkvT_c = np.ascontiguousarray(w_c.T)             # [H, 1920]
        woT_c = np.ascontiguousarray(wo[:, h0 * HD:h1 * HD].T.astype(wo_np_dt))
        maskT_c = np.ascontiguousarray(
            mask[h0:h1].transpose(0, 2, 1).astype(mask_np_dt))    # [5, T, T]
        in_maps.append({"xT": xT, "wqkvT": wqkvT_c, "woT": woT_c, "maskT": maskT_c})
    return in_maps


def kernel(input_pos=None, end=None, hidden_states=None, attention_mask=None,
           W_pack=None, o_proj_w=None, k_cache=None, v_cache=None):
    # input_pos == arange(T) and end == T per the problem spec, so the KV
    # cache write is a full overwrite and the zero-filled caches never
    # contribute to the output — both are intentionally unused here.
    in_maps = make_in_maps(hidden_states, attention_mask, W_pack, o_proj_w)
    nc = _get_nc()
    res = run_bass_kernel_spmd(nc, in_maps, list(range(NCORES)))
    y = res.results[0]["y"].astype(np.float32)
    for c in range(1, NCORES):
        y = y + res.results[c]["y"]
    return y.reshape(1, T, H)
